# revision 5
# baseline (speedup 1.0000x reference)
"""Trainium2 Bass kernel for nn_Block_25572235281069 (tiny causal transformer block).

Self-contained: kernel(**inputs) takes FULL inputs, shards batch across 8
NeuronCores (data parallel), runs a fused Bass/Tile kernel per core, gathers.

Per-core design (batch-on-partitions attention):
  supertile = 2048 tokens = 256 batches, strips of 512 tokens.
  X -> PE-transpose -> feature-major -> row-tiled qkv matmul (K=32 x4 strips)
  -> PE-transpose to batch-major [128 batches, (t,kqv,h,d)] -> DVE broadcast-AP
  products + reduces for causal softmax attention -> PE-transpose back ->
  proj/ff1/ff2 matmuls (tile_position packed) with fused residuals -> natural.
"""
import sys

for _p in ("/opt/trn_rl_repo", "/root/.axon_site/_ro/trn_rl_repo"):
    if _p not in sys.path:
        sys.path.insert(0, _p)

import numpy as np

import concourse.bass as bass
import concourse.bacc as bacc
import concourse.tile as tile
from concourse import mybir
from concourse import bass_utils
from concourse.bass import ds
from contextlib import ExitStack

FP = mybir.dt.float32
AX = mybir.AxisListType
OP = mybir.AluOpType
AF = mybir.ActivationFunctionType

C, T, H, D = 32, 8, 4, 8
SCALE = C ** -0.5
WCOLS = 480
N_CORES = 8
ST = 2048


def build_weight_blob(W_attn, W_proj, W_ff1, W_ff2):
    W_attn = np.asarray(W_attn); W_proj = np.asarray(W_proj)
    W_ff1 = np.asarray(W_ff1); W_ff2 = np.asarray(W_ff2)
    qkv = np.zeros((C, 96), np.float32)
    for kqv in range(3):
        for h in range(H):
            for d in range(D):
                qkv[:, kqv * 32 + h * 8 + d] = W_attn[h, :, kqv * 8 + d]
    blob = np.zeros((128, WCOLS), np.float32)
    for s in range(4):
        blob[32 * s:32 * s + 32, 0:96] = qkv
        blob[32 * s:32 * s + 32, 96:128] = W_proj
        blob[32 * s:32 * s + 32, 128:256] = W_ff1
    blob[:, 256:288] = W_ff2
    blob[:, 288:416] = np.eye(128, dtype=np.float32)
    m = np.tril(np.ones((T, T), np.float32)).reshape(64)
    blob[:, 416:480] = m[None, :]
    return blob


def apv(tile_ap, p0, pn, free_dims, foff=0):
    base = tile_ap[:] if not isinstance(tile_ap, bass.AP) else tile_ap
    ps = base.ap[0][0]
    return bass.AP(tensor=base.tensor, offset=base.offset + p0 * ps + foff,
                   ap=[[ps, pn]] + [list(x) for x in free_dims])


def emit_supertile(nc, pools, wsb, x_dram, o_dram, tok0):
    G, SS, NBT = 4, 512, 2
    w_qkv, w_proj = wsb[:, 0:96], wsb[:, 96:128]
    w_ff1, w_ff2 = wsb[:, 128:256], wsb[:, 256:288]
    ident = wsb[:, 288:416]

    x_nats = []
    for g in range(G):
        x_nat = pools["sb_nat"].tile([128, 4, 32], FP, tag="nat", name=f"x_nat{g}")
        srcg = bass.AP(tensor=x_dram.tensor,
                       offset=x_dram.offset + tok0 * 32 + g * 128 * 32,
                       ap=[[32, 128], [SS * 32, 4], [1, 32]])
        nc.sync.dma_start(out=x_nat, in_=srcg)
        x_nats.append(x_nat)

    xfm_ps = pools["ps_b"].tile([128, G, 128], FP, tag="b1", name="xfm_ps")
    for g in range(G):
        nc.tensor.transpose(xfm_ps[:, g, :], apv(x_nats[g], 0, 128, [[1, 128]]), ident)
    xfm = pools["sb_fm"].tile([128, G, 128], FP, tag="xfm", name="xfm")
    nc.scalar.copy(out=xfm[:], in_=xfm_ps[:])

    qkv_ps = [pools["ps_big"].tile([96, SS], FP, tag="big", name=f"qkv_ps{i}")
              for i in range(4)]
    for s in range(4):
        nc.tensor.matmul(qkv_ps[s][:], w_qkv[ds(32 * s, 32), :],
                         apv(xfm, 32 * s, 32, [[1, SS]]),
                         start=True, stop=True, tile_position=(32 * s, 0))
    qkv_sb = pools["sb_qkv"].tile([96, 4, 8, 64], FP, tag="qkv", name="qkv_sb")
    for s in range(4):
        src_v = apv(qkv_ps[s], 0, 96, [[1, 8], [8, 64]])
        if s % 2 == 0:
            nc.scalar.copy(out=qkv_sb[:, s, :, :], in_=src_v)
        else:
            nc.vector.tensor_copy(out=qkv_sb[:, s, :, :], in_=src_v)

    bp_sbs = []
    for bt in range(NBT):
        bp_ps = [pools["ps_bp"].tile([64, 4, 96], FP, tag="bp", name=f"bp_ps{bt}_{i}")
                 for i in range(4)]
        for half in range(2):
            for tt in range(4):
                t = half * 4 + tt
                for sh in range(2):
                    s = 2 * bt + sh
                    nc.tensor.transpose(
                        apv(bp_ps[half * 2 + sh], 0, 64, [[1, 96]], tt * 96),
                        apv(qkv_sb, 0, 96, [[1, 64]], s * SS + t * 64),
                        ident[0:96, 0:96])
        bp = pools["sb_bp"].tile([128, 8, 96], FP, tag="bp", name=f"bp{bt}")
        for half in range(2):
            for sh in range(2):
                dst_v = bp[64 * sh:64 * sh + 64, 4 * half:4 * half + 4, :]
                if (half + sh) % 2 == 0:
                    nc.scalar.copy(out=dst_v, in_=bp_ps[half * 2 + sh][:])
                else:
                    nc.vector.tensor_copy(out=dst_v, in_=bp_ps[half * 2 + sh][:])
        bp_sbs.append(bp)

    attn_sbs = []
    for bt in range(NBT):
        bp = bp_sbs[bt]
        # P layout (i, j, h, d); Q/K iter (i, j, hd-merged)
        P = pools["sb_big"].tile([128, 2048], FP, tag="P", name=f"P{bt}")
        nc.vector.tensor_mul(
            P[:],
            apv(bp, 0, 128, [[96, 8], [0, 8], [1, 32]], 32),
            apv(bp, 0, 128, [[0, 8], [96, 8], [1, 32]], 0))
        # S layout (i, j, h)
        S = pools["sb_sm"].tile([128, 256], FP, tag="S", name=f"S{bt}")
        nc.vector.tensor_reduce(
            out=S[:], in_=apv(P, 0, 128, [[8, 256], [1, 8]]),
            axis=AX.X, op=OP.add)
        E = pools["sb_sm"].tile([128, 256], FP, tag="E", name=f"E{bt}")
        nc.scalar.activation(out=E[:], in_=S[:], func=AF.Exp, scale=SCALE)
        nc.vector.tensor_mul(
            E[:], E[:], apv(wsb, 0, 128, [[8, 8], [1, 8], [0, 4]], 416))
        # den (i, h) via j-reduce (strided inner)
        den = pools["sb_sm"].tile([128, 32], FP, tag="den", name=f"den{bt}")
        nc.vector.tensor_reduce(
            out=den[:], in_=apv(E, 0, 128, [[32, 8], [1, 4], [4, 8]]),
            axis=AX.X, op=OP.add)
        rden = pools["sb_sm"].tile([128, 32], FP, tag="rden", name=f"rden{bt}")
        nc.vector.reciprocal(out=rden[:], in_=den[:])
        # AV per head: AVP_h [128, (i, d, j)]; att_u (i, h, d)
        att_u = pools["sb_sm"].tile([128, 256], FP, tag="attu", name=f"attu{bt}")
        for h in range(4):
            AVP = pools["sb_big"].tile([128, 512], FP, tag="AVP", name=f"AVP{bt}_{h}")
            nc.vector.tensor_mul(
                AVP[:],
                apv(E, 0, 128, [[32, 8], [0, 8], [4, 8]], h),
                apv(bp, 0, 128, [[0, 8], [1, 8], [96, 8]], 64 + 8 * h))
            nc.vector.tensor_reduce(
                out=apv(att_u, 0, 128, [[32, 8], [1, 8]], 8 * h),
                in_=apv(AVP, 0, 128, [[8, 64], [1, 8]]),
                axis=AX.X, op=OP.add)
        attn = pools["sb_sm"].tile([128, 256], FP, tag="attn", name=f"attn{bt}")
        nc.vector.tensor_mul(
            attn[:], att_u[:], apv(rden, 0, 128, [[4, 8], [1, 4], [0, 8]]))
        attn_sbs.append(attn)

    afm_pss = [pools["ps_bp"].tile([32, 8, 64], FP, tag="bp", name=f"afm_ps{i}")
               for i in range(4)]
    for s in range(4):
        bt, sh = s // 2, s % 2
        for t in range(8):
            nc.tensor.transpose(
                apv(afm_pss[s], 0, 32, [[1, 64]], t * 64),
                apv(attn_sbs[bt], 64 * sh, 64, [[1, 32]], t * 32),
                ident[64 * sh:64 * sh + 64, 64 * sh:64 * sh + 64])
    afm = pools["sb_fm"].tile([128, SS], FP, tag="afm", name="afm")
    for s in range(4):
        src_v = apv(afm_pss[s], 0, 32, [[1, 64], [64, 8]])
        if s % 2 == 0:
            nc.scalar.copy(out=afm[32 * s:32 * s + 32, :], in_=src_v)
        else:
            nc.vector.tensor_copy(out=afm[32 * s:32 * s + 32, :], in_=src_v)

    proj_ps = pools["ps_b"].tile([128, SS], FP, tag="b1", name="proj_ps")
    for s in range(4):
        nc.tensor.matmul(proj_ps[ds(32 * s, 32), :], w_proj[ds(32 * s, 32), :],
                         apv(afm, 32 * s, 32, [[1, SS]]),
                         start=True, stop=True, tile_position=(32 * s, 32 * s))
    h1 = pools["sb_fm"].tile([128, SS], FP, tag="h1", name="h1")
    nc.vector.tensor_add(h1[:], proj_ps[:], apv(xfm, 0, 128, [[1, SS]]))

    ff1_ps = [pools["ps_big"].tile([128, SS], FP, tag="big", name=f"ff1_ps{i}")
              for i in range(4)]
    for s in range(4):
        nc.tensor.matmul(ff1_ps[s][:], w_ff1[ds(32 * s, 32), :],
                         apv(h1, 32 * s, 32, [[1, SS]]),
                         start=True, stop=True, tile_position=(32 * s, 0))
    hid = pools["sb_hid"].tile([128, 4, SS], FP, tag="hid", name="hid")
    for s in range(4):
        nc.scalar.activation(out=hid[:, s, :], in_=ff1_ps[s][:], func=AF.Relu)

    ff2_ps = pools["ps_b"].tile([128, SS], FP, tag="b1", name="ff2_ps")
    for s in range(4):
        nc.tensor.matmul(ff2_ps[ds(32 * s, 32), :], w_ff2[:, :], hid[:, s, :],
                         start=True, stop=True, tile_position=(0, 32 * s))
    ofm = pools["sb_fm"].tile([128, SS], FP, tag="ofm", name="ofm")
    nc.vector.tensor_add(ofm[:], h1[:], ff2_ps[:])

    onat_ps = pools["ps_b"].tile([128, G, 4, 32], FP, tag="b1", name="onat_ps")
    for g in range(G):
        nc.tensor.transpose(
            apv(onat_ps, 0, 128, [[1, 128]], g * 128),
            apv(ofm, 0, 128, [[1, 128]], 128 * g),
            ident)
    onat = pools["sb_nat"].tile([128, 4, G, 32], FP, tag="onat", name="onat")
    nc.scalar.copy(out=onat[:],
                   in_=apv(onat_ps, 0, 128, [[32, 4], [128, G], [1, 32]]))

    dst = bass.AP(tensor=o_dram.tensor, offset=o_dram.offset + tok0 * 32,
                  ap=[[32, 128], [SS * 32, 4], [128 * 32, G], [1, 32]])
    nc.sync.dma_start(out=dst, in_=onat[:])


def build_kernel(ntok_per_core):
    assert ntok_per_core % ST == 0
    nsuper = ntok_per_core // ST
    nc = bacc.Bacc("TRN2", target_bir_lowering=False, debug=False)
    xd = nc.dram_tensor("X", (ntok_per_core, 32), FP, kind="ExternalInput")
    wd = nc.dram_tensor("WB", (128, WCOLS), FP, kind="ExternalInput")
    od = nc.dram_tensor("O", (ntok_per_core, 32), FP, kind="ExternalOutput")
    with tile.TileContext(nc) as tc:
        with ExitStack() as ctx:
            pools = {}
            pools["ps_b"] = ctx.enter_context(tc.tile_pool(name="ps_b", bufs=2, space="PSUM"))
            pools["ps_big"] = ctx.enter_context(tc.tile_pool(name="ps_big", bufs=4, space="PSUM"))
            pools["ps_bp"] = ctx.enter_context(tc.tile_pool(name="ps_bp", bufs=2, space="PSUM"))
            for nm, bufs in [("singles", 1), ("sb_nat", 2), ("sb_fm", 2), ("sb_qkv", 2),
                             ("sb_bp", 2), ("sb_big", 2), ("sb_sm", 2), ("sb_hid", 2)]:
                pools[nm] = ctx.enter_context(tc.tile_pool(name=nm, bufs=bufs))
            wsb = pools["singles"].tile([128, WCOLS], FP, name="wsb")
            nc.sync.dma_start(out=wsb, in_=wd[:])
            for it in range(nsuper):
                emit_supertile(nc, pools, wsb, xd[:], od[:], it * ST)
    nc.compile()
    return nc


_CACHE = {}


def kernel(X, W_attn, W_proj, W_ff1, W_ff2):
    X = np.ascontiguousarray(np.asarray(X), dtype=np.float32)
    b, t, c = X.shape
    ntok = b * t
    per_core = ntok // N_CORES
    blob = build_weight_blob(W_attn, W_proj, W_ff1, W_ff2)

    if per_core not in _CACHE:
        _CACHE[per_core] = build_kernel(per_core)
    nc = _CACHE[per_core]

    Xf = X.reshape(ntok, 32)
    in_maps = [{"X": np.ascontiguousarray(Xf[i * per_core:(i + 1) * per_core]),
                "WB": blob} for i in range(N_CORES)]
    res = bass_utils.run_bass_kernel_spmd(nc, in_maps, core_ids=list(range(N_CORES)))
    out = np.concatenate([res.results[i]["O"] for i in range(N_CORES)], axis=0)
    return out.reshape(b, t, c).astype(np.float32)


if __name__ == "__main__":
    rng = np.random.RandomState(0)
    b = 2048 * 8 // 8
    X = rng.randn(2048, 8, 32).astype(np.float32)
    W_attn = (rng.randn(4, 32, 24) * 0.02).astype(np.float32)
    W_proj = (rng.randn(32, 32) * 0.02).astype(np.float32)
    W_ff1 = (rng.randn(32, 128) * 0.02).astype(np.float32)
    W_ff2 = (rng.randn(128, 32) * 0.02).astype(np.float32)
    out = kernel(X=X, W_attn=W_attn, W_proj=W_proj, W_ff1=W_ff1, W_ff2=W_ff2)
    print("out", out.shape, out.dtype)
